# revision 16
# baseline (speedup 1.0000x reference)
"""ExpanderGIN message-passing kernel for 8 Trainium2 NeuronCores.

out = relu((x + segment_sum(x[src], dst)) @ W.T + b)

Strategy (graph-parallel, no collectives), v2 — bf16 datapath:
  - Destination nodes are sharded 8 ways (12500 nodes/core -> 98 tiles of
    128 slots, degree-balanced serpentine assignment). x is replicated
    per core as a bf16 table; edge rows are fetched with SWDGE dma_gather
    (256B bf16 rows). The int16 index limit is handled with 4
    quarter-tables of 25000 rows.
  - The per-core edge list is laid out group-major (14 tiles per group),
    so each (group, quarter) is ONE large gather instruction (~3.5K
    indices) on a rotating SWDGE queue: descriptor generation (994ns
    fixed + 0.34ns/desc, serialized on the Pool engine) was the v1
    bottleneck at ~200 instructions/core.
  - Aggregation: per 128-edge chunk a one-hot(dst) [128 edge, 128 slot]
    bf16 matrix is built on DVE (iota vs dst compare, one instruction
    per (group, quarter)), then TensorE accumulates
    psum[feat, slot] += gx^T @ onehot in f32 PSUM. bf16 matmuls run at
    1 cycle/row vs 4 for f32 (v1's second bottleneck).
  - The self term x is folded into the same PSUM via one extra matmul
    per tile: lhsT = x rows of the tile (slot-major), rhs = 128x128
    identity.
  - PSUM eviction (cast to bf16 ht) runs on the Scalar engine, keeping
    DVE free for one-hot builds. MLP: po[slot, outfeat] = ht.T @ W^T
    plus a K=1 ones x bias matmul; ReLU on Scalar engine writes a
    grouped [slot, tile*feat] bf16 tile, stored once per group (4KB
    HBM lines keep HWDGE descriptor count low).
  - Host converts the bf16 output back to f32 (grading tolerance 2e-2;
    measured rel err ~4e-3).
"""

import numpy as np
import ml_dtypes

N = 100000
E = 625000
D = 128
NC = 8            # cores
NPC = N // NC     # 12500 nodes per core
P = 128
TPC = (NPC + P - 1) // P   # 98 tiles per core
SLOTS = TPC * P            # 12544 slots per core
NQ = 4                     # quarter tables (int16 index limit)
QROWS = N // NQ            # 25000
G = 14                     # tiles per gather group (98 = 7 groups of 14)
NG = TPC // G

_f32 = np.float32
_bf16 = ml_dtypes.bfloat16


def _assign_quarters(src, ct):
    """Greedy source-node -> quarter assignment to minimize gather padding.

    Btq[t, q] = ceil(max_c cnt[c,t,q]/128) blocks are allocated per
    (tile, quarter) bucket; uniform quarters cost 2 blocks each (mean
    count ~199), 8 blocks/tile where ceil(797/128) = 7 is the floor.
    Quarter 3 is kept small (every bucket <= 1 block at every core) and
    quarters 0-2 are clamped just under the 2-block boundary: 2+2+2+1 =
    7 blocks for nearly all tiles.
    """
    cap = 32768  # int16 gather index limit
    order_edges = np.argsort(src, kind="stable")
    s_sorted = src[order_edges]
    ct_sorted = ct[order_edges]
    starts = np.searchsorted(s_sorted, np.arange(N + 1))
    deg = starts[1:] - starts[:-1]
    node_order = np.argsort(-deg, kind="stable")

    cnt = np.zeros((NQ, NC * TPC), np.int32)
    sizes = np.zeros(NQ, np.int64)
    qof = np.zeros(N, np.int8)
    cap_cnt = np.array([254, 254, 254, 126], np.int32)

    for n in node_order:
        a, b = starts[n], starts[n + 1]
        if a == b:
            q = int(np.argmin(sizes[:3]))
            qof[n] = q
            sizes[q] += 1
            continue
        u, m = np.unique(ct_sorted[a:b], return_counts=True)
        placed = False
        for q in (3, *np.argsort(sizes[:3])):
            if sizes[q] >= cap:
                continue
            if np.all(cnt[q, u] + m <= cap_cnt[q]):
                cnt[q, u] += m
                sizes[q] += 1
                qof[n] = q
                placed = True
                break
        if not placed:
            best_q, best_over = 0, None
            for q in range(3):
                if sizes[q] >= cap:
                    continue
                over = int(np.maximum(cnt[q, u] + m - cap_cnt[q], 0).sum())
                if best_over is None or over < best_over:
                    best_over, best_q = over, q
            q = best_q
            cnt[q, u] += m
            sizes[q] += 1
            qof[n] = q

    pos = np.zeros(N, np.int64)
    offs = np.zeros(NQ, np.int64)
    for n in range(N):
        q = qof[n]
        pos[n] = offs[q]
        offs[q] += 1
    assert offs.max() <= cap
    return qof.astype(np.int64), pos, offs


def _preprocess(edge_index):
    """Shard edges. Returns per-core host arrays + layout metadata."""
    src = np.asarray(edge_index[0]).astype(np.int64)
    dst = np.asarray(edge_index[1]).astype(np.int64)
    deg = np.bincount(dst, minlength=N)

    # serpentine degree-balanced node -> slot assignment per core
    node_of = np.full((NC, SLOTS), -1, np.int64)   # slot -> global node
    slot_of = np.empty(N, np.int64)                # global node -> slot (in its core)
    for c in range(NC):
        nodes = np.arange(c * NPC, (c + 1) * NPC)
        order = nodes[np.argsort(-deg[nodes], kind="stable")]
        padded = np.concatenate([order, np.full(SLOTS - NPC, -1, np.int64)])
        arr = padded.reshape(P, TPC).copy()
        arr[1::2] = arr[1::2, ::-1]
        node_of[c] = arr.T.reshape(-1)
        m = node_of[c] >= 0
        slot_of[node_of[c][m]] = np.nonzero(m)[0]

    ec = dst // NPC
    eslot = slot_of[dst]
    et = eslot // P
    epos = (eslot % P).astype(_f32)
    # greedy source->quarter assignment: shapes per-(tile,quarter) counts
    # so most tiles need 7 gather blocks instead of 8 (-12.5% gather S)
    qof, qpos, qsizes = _assign_quarters(src, ec * TPC + et)
    qstart = np.concatenate([[0], np.cumsum(qsizes)])[:-1]
    eq = qof[src]
    eqidx = qpos[src].astype(np.int16)
    # device x table is permuted: quarter q occupies rows
    # [qstart[q], qstart[q]+qsizes[q]); row qstart[qof[n]]+qpos[n] = x[n]
    xrow_of_node = qstart[qof] + qpos

    # counts per (core, tile, quarter); block counts = max over cores
    key = (ec * TPC + et) * NQ + eq
    cnt = np.bincount(key, minlength=NC * TPC * NQ).reshape(NC, TPC, NQ)
    Btq = ((cnt.max(axis=0) + P - 1) // P).astype(np.int64)  # [TPC, NQ]

    groups = [(g * G, (g + 1) * G) for g in range(NG)]

    # slot layout: nest group -> q -> t in group -> blocks
    slot_start = np.zeros((TPC, NQ), np.int64)
    pos = 0
    for (ta, tb) in groups:
        for q in range(NQ):
            for t in range(ta, tb):
                slot_start[t, q] = pos
                pos += Btq[t, q] * P
    S_total = pos
    assert S_total % 128 == 0

    # rank of each edge within its (c,t,q) group
    perm = np.argsort(key, kind="stable")
    gstart = np.concatenate([[0], np.cumsum(np.bincount(key, minlength=NC * TPC * NQ))])[:-1]
    ranks = np.empty(len(perm), np.int64)
    ranks[perm] = np.arange(len(perm)) - gstart[key[perm]]

    flat = slot_start[et, eq] + ranks   # slot within core's flat layout

    qidx_slots = np.zeros((NC, S_total), np.int16)
    dst_slots = np.full((NC, S_total), 200.0, _f32)
    qidx_slots[ec, flat] = eqidx
    dst_slots[ec, flat] = epos

    idx16 = np.empty((NC, P, S_total // 16), np.int16)
    dstl = np.empty((NC, P, S_total // 128), _f32)
    for c in range(NC):
        wrapped = qidx_slots[c].reshape(-1, 16).T   # [16, S/16]
        idx16[c] = np.tile(wrapped, (8, 1))
        dstl[c] = dst_slots[c].reshape(-1, 128).T   # [128, S/128]

    return {
        "Btq": Btq,
        "slot_start": slot_start,
        "S_total": S_total,
        "groups": groups,
        "idx16": idx16,
        "dstl": dstl,
        "node_of": node_of,
        "qstart": qstart,
        "qsizes": qsizes,
        "xrow_of_node": xrow_of_node,
    }


def _build_program(Btq, slot_start, S_total, groups, qstart, qsizes, repeat=1):
    import concourse.bacc as bacc
    import concourse.mybir as mybir
    import concourse.tile as tile
    from contextlib import ExitStack

    f32 = mybir.dt.float32
    bf16 = mybir.dt.bfloat16
    nc = bacc.Bacc(
        "TRN2", target_bir_lowering=False, debug=False, num_devices=NC,
        num_swdge_queues=4,
    )

    x_d = nc.dram_tensor("x", [N, D], bf16, kind="ExternalInput")
    xg_d = nc.dram_tensor("xg", [NG * P, G * D], bf16, kind="ExternalInput")
    idx_d = nc.dram_tensor("idx16", [P, S_total // 16], mybir.dt.int16, kind="ExternalInput")
    dst_d = nc.dram_tensor("dstl", [P, S_total // 128], f32, kind="ExternalInput")
    wt_d = nc.dram_tensor("wt", [D, D], bf16, kind="ExternalInput")
    b_d = nc.dram_tensor("bias", [1, D], bf16, kind="ExternalInput")
    out_d = nc.dram_tensor("out", [NG * P, G * D], bf16, kind="ExternalOutput")

    with tile.TileContext(nc) as tc, ExitStack() as ctx:
        const = ctx.enter_context(tc.tile_pool(name="const", bufs=1))
        gxp = ctx.enter_context(tc.tile_pool(name="gx", bufs=3))
        ohp = ctx.enter_context(tc.tile_pool(name="oh", bufs=3))
        xgp = ctx.enter_context(tc.tile_pool(name="xg", bufs=3))
        htp = ctx.enter_context(tc.tile_pool(name="ht", bufs=4))
        obp = ctx.enter_context(tc.tile_pool(name="ob", bufs=3))
        pag = ctx.enter_context(tc.tile_pool(name="pagg", bufs=4, space="PSUM"))
        pou = ctx.enter_context(tc.tile_pool(name="pout", bufs=4, space="PSUM"))

        idx_t = const.tile([P, S_total // 16], mybir.dt.int16)
        nc.sync.dma_start(out=idx_t[:], in_=idx_d[:])
        dst_t = const.tile([P, S_total // 128], f32)
        nc.sync.dma_start(out=dst_t[:], in_=dst_d[:])
        wt_t = const.tile([D, D], bf16)
        nc.sync.dma_start(out=wt_t[:], in_=wt_d[:])
        b_t = const.tile([1, D], bf16)
        nc.sync.dma_start(out=b_t[:], in_=b_d[:])
        ones_t = const.tile([1, D], bf16)
        nc.vector.memset(ones_t[:], 1.0)
        iota_i = const.tile([P, P], mybir.dt.int32)
        nc.gpsimd.iota(iota_i[:], pattern=[[1, P]], base=0, channel_multiplier=0)
        iota_f = const.tile([P, P], bf16)
        nc.vector.tensor_copy(out=iota_f[:], in_=iota_i[:])
        iota_p = const.tile([P, 1], mybir.dt.int32)
        nc.gpsimd.iota(iota_p[:], pattern=[[1, 1]], base=0, channel_multiplier=1)
        ident_t = const.tile([P, P], bf16)
        nc.vector.tensor_tensor(
            out=ident_t[:],
            in0=iota_i[:],
            in1=iota_p[:].to_broadcast([P, P]),
            op=mybir.AluOpType.is_equal,
        )

        gather_count = 0
        for _rep in range(repeat):
            for gi, (ta, tb) in enumerate(groups):
                Bg = int(Btq[ta:tb, :].sum())
                c0g = int(slot_start[ta, 0]) // P  # first chunk col of group
                gx = gxp.tile([P, Bg, P], bf16, tag="gx")
                oh = ohp.tile([P, Bg, P], bf16, tag="oh")
                for q in range(NQ):
                    Bgq = int(Btq[ta:tb, q].sum())
                    if Bgq == 0:
                        continue
                    c0 = int(slot_start[ta, q]) // P
                    b0 = c0 - c0g
                    # HW ring limit: 1024 indices (8 blocks) per dma_gather
                    for sb in range(0, Bgq, 8):
                        nb = min(8, Bgq - sb)
                        nidx = nb * P
                        nc.gpsimd.dma_gather(
                            gx[:, b0 + sb : b0 + sb + nb, :],
                            x_d[int(qstart[q]) : int(qstart[q] + qsizes[q]), :],
                            idx_t[:, (c0 + sb) * 8 : (c0 + sb) * 8 + nidx // 16],
                            nidx,
                            nidx,
                            D,
                            queue_num=gather_count % 4,
                        )
                        gather_count += 1
                    # one-hot per chunk via tensor_scalar (per-partition dst
                    # scalar) — keeps innermost strides 1 so DVE 2x/4x perf
                    # modes engage, unlike a broadcast tensor_tensor
                    for bb in range(Bgq):
                        nc.vector.tensor_scalar(
                            out=oh[:, b0 + bb, :],
                            in0=iota_f[:],
                            scalar1=dst_t[:, c0 + bb : c0 + bb + 1],
                            scalar2=None,
                            op0=mybir.AluOpType.is_equal,
                        )
                xg_t = xgp.tile([P, G * D], bf16, tag="xg")
                nc.sync.dma_start(out=xg_t[:], in_=xg_d[gi * P : (gi + 1) * P, :])
                ob = obp.tile([P, G * D], bf16, tag="ob")
                for i, t in enumerate(range(ta, tb)):
                    psum = pag.tile([P, P], f32, space="PSUM", tag="pagg")
                    # self term: psum[feat, slot] = x_tile^T via identity
                    nc.tensor.matmul(
                        out=psum[:],
                        lhsT=xg_t[:, i * D : (i + 1) * D],
                        rhs=ident_t[:],
                        start=True,
                        stop=False,
                    )
                    chunks = [(q, b) for q in range(NQ) for b in range(int(Btq[t, q]))]
                    for j, (q, b) in enumerate(chunks):
                        boff = (int(slot_start[t, q]) // P) - c0g + b
                        nc.tensor.matmul(
                            out=psum[:],
                            lhsT=gx[:, boff, :],
                            rhs=oh[:, boff, :],
                            start=False,
                            stop=(j == len(chunks) - 1),
                        )
                    ht = htp.tile([P, P], bf16, tag="ht")
                    nc.scalar.activation(ht[:], psum[:], mybir.ActivationFunctionType.Copy)
                    po = pou.tile([P, P], f32, space="PSUM", tag="pout")
                    nc.tensor.matmul(out=po[:], lhsT=ht[:], rhs=wt_t[:], start=True, stop=False)
                    nc.tensor.matmul(out=po[:], lhsT=ones_t[:], rhs=b_t[:], start=False, stop=True)
                    nc.scalar.activation(
                        ob[:, i * D : (i + 1) * D], po[:],
                        mybir.ActivationFunctionType.Relu,
                    )
                nc.sync.dma_start(out=out_d[gi * P : (gi + 1) * P, :], in_=ob[:])
    nc.compile()
    return nc


def _prepare(x, edge_index, W, b, repeat=1):
    x = np.asarray(x, dtype=_f32)
    W = np.asarray(W, dtype=_f32)
    b = np.asarray(b, dtype=_f32)
    pre = _preprocess(edge_index)
    nc = _build_program(
        pre["Btq"], pre["slot_start"], pre["S_total"], pre["groups"],
        pre["qstart"], pre["qsizes"], repeat=repeat,
    )
    x_bf16_nat = x.astype(_bf16)
    # device x table is permuted so quarter q is a contiguous row range
    x_bf = np.empty((N, D), _bf16)
    x_bf[pre["xrow_of_node"]] = x_bf16_nat
    wt = np.ascontiguousarray(W.T.astype(_bf16))
    brow = np.ascontiguousarray(b.reshape(1, D).astype(_bf16))
    node_of = pre["node_of"]
    in_maps = []
    for c in range(NC):
        nidx = np.where(node_of[c] < 0, 0, node_of[c])
        # xg[g*128 + slot, i*128 + feat] = x[node at tile (g*G+i), slot]
        xg = np.ascontiguousarray(
            x_bf16_nat[nidx]               # [SLOTS, D]
            .reshape(NG, G, P, D)          # [g, i, slot, feat]
            .transpose(0, 2, 1, 3)         # [g, slot, i, feat]
            .reshape(NG * P, G * D)
        )
        in_maps.append(
            {
                "x": x_bf,
                "xg": xg,
                "idx16": np.ascontiguousarray(pre["idx16"][c]),
                "dstl": np.ascontiguousarray(pre["dstl"][c]),
                "wt": wt,
                "bias": brow,
            }
        )
    return nc, in_maps, node_of


def _assemble(results, node_of):
    out = np.empty((N, D), _f32)
    for c in range(NC):
        oc = (
            np.asarray(results[c]["out"])
            .astype(_f32)
            .reshape(NG, P, G, D)
            .transpose(0, 2, 1, 3)      # [g, i, slot, feat]
            .reshape(SLOTS, D)
        )
        m = node_of[c] >= 0
        out[node_of[c][m]] = oc[m]
    return out


def kernel(x, edge_index, W, b):
    from concourse.bass_utils import run_bass_kernel_spmd

    nc, in_maps, node_of = _prepare(x, edge_index, W, b)
    res = run_bass_kernel_spmd(nc, in_maps, core_ids=list(range(NC)))
    return _assemble(res.results, node_of)


# revision 17
# speedup vs baseline: 1.1316x; 1.1316x over previous
"""ExpanderGIN message-passing kernel for 8 Trainium2 NeuronCores.

out = relu((x + segment_sum(x[src], dst)) @ W.T + b)

Strategy (graph-parallel, no collectives), v2 — bf16 datapath:
  - Destination nodes are sharded 8 ways (12500 nodes/core -> 98 tiles of
    128 slots, degree-balanced serpentine assignment). x is replicated
    per core as a bf16 table; edge rows are fetched with SWDGE dma_gather
    (256B bf16 rows). The int16 index limit is handled with 4
    quarter-tables of 25000 rows.
  - The per-core edge list is laid out group-major (14 tiles per group),
    so each (group, quarter) is ONE large gather instruction (~3.5K
    indices) on a rotating SWDGE queue: descriptor generation (994ns
    fixed + 0.34ns/desc, serialized on the Pool engine) was the v1
    bottleneck at ~200 instructions/core.
  - Aggregation: per 128-edge chunk a one-hot(dst) [128 edge, 128 slot]
    bf16 matrix is built on DVE (iota vs dst compare, one instruction
    per (group, quarter)), then TensorE accumulates
    psum[feat, slot] += gx^T @ onehot in f32 PSUM. bf16 matmuls run at
    1 cycle/row vs 4 for f32 (v1's second bottleneck).
  - The self term x is folded into the same PSUM via one extra matmul
    per tile: lhsT = x rows of the tile (slot-major), rhs = 128x128
    identity.
  - PSUM eviction (cast to bf16 ht) runs on the Scalar engine, keeping
    DVE free for one-hot builds. MLP: po[slot, outfeat] = ht.T @ W^T
    plus a K=1 ones x bias matmul; ReLU on Scalar engine writes a
    grouped [slot, tile*feat] bf16 tile, stored once per group (4KB
    HBM lines keep HWDGE descriptor count low).
  - Host converts the bf16 output back to f32 (grading tolerance 2e-2;
    measured rel err ~4e-3).
"""

import numpy as np
import ml_dtypes

N = 100000
E = 625000
D = 128
NC = 8            # cores
NPC = N // NC     # 12500 nodes per core
P = 128
TPC = (NPC + P - 1) // P   # 98 tiles per core
SLOTS = TPC * P            # 12544 slots per core
NQ = 4                     # quarter tables (int16 index limit)
QROWS = N // NQ            # 25000
G = 14                     # tiles per gather group (98 = 7 groups of 14)
NG = TPC // G

_f32 = np.float32
_bf16 = ml_dtypes.bfloat16


def _assign_quarters(src, ct):
    """Greedy source-node -> quarter assignment to minimize gather padding.

    Btq[t, q] = ceil(max_c cnt[c,t,q]/128) blocks are allocated per
    (tile, quarter) bucket; uniform quarters cost 2 blocks each (mean
    count ~199), 8 blocks/tile where ceil(797/128) = 7 is the floor.
    Quarter 3 is kept small (every bucket <= 1 block at every core) and
    quarters 0-2 are clamped just under the 2-block boundary: 2+2+2+1 =
    7 blocks for nearly all tiles.
    """
    cap = 32768  # int16 gather index limit
    order_edges = np.argsort(src, kind="stable")
    s_sorted = src[order_edges]
    ct_sorted = ct[order_edges]
    starts = np.searchsorted(s_sorted, np.arange(N + 1))
    deg = starts[1:] - starts[:-1]
    node_order = np.argsort(-deg, kind="stable")

    cnt = np.zeros((NQ, NC * TPC), np.int32)
    sizes = np.zeros(NQ, np.int64)
    qof = np.zeros(N, np.int8)
    cap_cnt = np.array([254, 254, 254, 126], np.int32)

    for n in node_order:
        a, b = starts[n], starts[n + 1]
        if a == b:
            q = int(np.argmin(sizes[:3]))
            qof[n] = q
            sizes[q] += 1
            continue
        u, m = np.unique(ct_sorted[a:b], return_counts=True)
        placed = False
        for q in (3, *np.argsort(sizes[:3])):
            if sizes[q] >= cap:
                continue
            if np.all(cnt[q, u] + m <= cap_cnt[q]):
                cnt[q, u] += m
                sizes[q] += 1
                qof[n] = q
                placed = True
                break
        if not placed:
            best_q, best_over = 0, None
            for q in range(3):
                if sizes[q] >= cap:
                    continue
                over = int(np.maximum(cnt[q, u] + m - cap_cnt[q], 0).sum())
                if best_over is None or over < best_over:
                    best_over, best_q = over, q
            q = best_q
            cnt[q, u] += m
            sizes[q] += 1
            qof[n] = q

    pos = np.zeros(N, np.int64)
    offs = np.zeros(NQ, np.int64)
    for n in range(N):
        q = qof[n]
        pos[n] = offs[q]
        offs[q] += 1
    assert offs.max() <= cap
    return qof.astype(np.int64), pos, offs


def _preprocess(edge_index):
    """Shard edges. Returns per-core host arrays + layout metadata."""
    src = np.asarray(edge_index[0]).astype(np.int64)
    dst = np.asarray(edge_index[1]).astype(np.int64)
    deg = np.bincount(dst, minlength=N)

    # serpentine degree-balanced node -> slot assignment per core
    node_of = np.full((NC, SLOTS), -1, np.int64)   # slot -> global node
    slot_of = np.empty(N, np.int64)                # global node -> slot (in its core)
    for c in range(NC):
        nodes = np.arange(c * NPC, (c + 1) * NPC)
        order = nodes[np.argsort(-deg[nodes], kind="stable")]
        padded = np.concatenate([order, np.full(SLOTS - NPC, -1, np.int64)])
        arr = padded.reshape(P, TPC).copy()
        arr[1::2] = arr[1::2, ::-1]
        node_of[c] = arr.T.reshape(-1)
        m = node_of[c] >= 0
        slot_of[node_of[c][m]] = np.nonzero(m)[0]

    ec = dst // NPC
    eslot = slot_of[dst]
    et = eslot // P
    epos = (eslot % P).astype(_f32)
    # greedy source->quarter assignment: shapes per-(tile,quarter) counts
    # so most tiles need 7 gather blocks instead of 8 (-12.5% gather S)
    qof, qpos, qsizes = _assign_quarters(src, ec * TPC + et)
    qstart = np.concatenate([[0], np.cumsum(qsizes)])[:-1]
    eq = qof[src]
    eqidx = qpos[src].astype(np.int16)
    # device x table is permuted: quarter q occupies rows
    # [qstart[q], qstart[q]+qsizes[q]); row qstart[qof[n]]+qpos[n] = x[n]
    xrow_of_node = qstart[qof] + qpos

    # counts per (core, tile, quarter); block counts = max over cores
    key = (ec * TPC + et) * NQ + eq
    cnt = np.bincount(key, minlength=NC * TPC * NQ).reshape(NC, TPC, NQ)
    Btq = ((cnt.max(axis=0) + P - 1) // P).astype(np.int64)  # [TPC, NQ]

    groups = [(g * G, (g + 1) * G) for g in range(NG)]

    # slot layout: nest group -> q -> t in group -> blocks
    slot_start = np.zeros((TPC, NQ), np.int64)
    pos = 0
    for (ta, tb) in groups:
        for q in range(NQ):
            for t in range(ta, tb):
                slot_start[t, q] = pos
                pos += Btq[t, q] * P
    S_total = pos
    assert S_total % 128 == 0

    # rank of each edge within its (c,t,q) group
    perm = np.argsort(key, kind="stable")
    gstart = np.concatenate([[0], np.cumsum(np.bincount(key, minlength=NC * TPC * NQ))])[:-1]
    ranks = np.empty(len(perm), np.int64)
    ranks[perm] = np.arange(len(perm)) - gstart[key[perm]]

    flat = slot_start[et, eq] + ranks   # slot within core's flat layout

    qidx_slots = np.zeros((NC, S_total), np.int16)
    dst_slots = np.full((NC, S_total), 200.0, _f32)
    qidx_slots[ec, flat] = eqidx
    dst_slots[ec, flat] = epos

    idx16 = np.empty((NC, P, S_total // 16), np.int16)
    dstl = np.empty((NC, P, S_total // 128), _f32)
    for c in range(NC):
        wrapped = qidx_slots[c].reshape(-1, 16).T   # [16, S/16]
        idx16[c] = np.tile(wrapped, (8, 1))
        dstl[c] = dst_slots[c].reshape(-1, 128).T   # [128, S/128]

    return {
        "Btq": Btq,
        "slot_start": slot_start,
        "S_total": S_total,
        "groups": groups,
        "idx16": idx16,
        "dstl": dstl,
        "node_of": node_of,
        "qstart": qstart,
        "qsizes": qsizes,
        "xrow_of_node": xrow_of_node,
    }


def _build_program(Btq, slot_start, S_total, groups, qstart, qsizes, repeat=1):
    import concourse.bacc as bacc
    import concourse.mybir as mybir
    import concourse.tile as tile
    from contextlib import ExitStack

    f32 = mybir.dt.float32
    bf16 = mybir.dt.bfloat16
    nc = bacc.Bacc(
        "TRN2", target_bir_lowering=False, debug=False, num_devices=NC,
        num_swdge_queues=4,
    )

    x_d = nc.dram_tensor("x", [N, D], bf16, kind="ExternalInput")
    xg_d = nc.dram_tensor("xg", [NG * P, G * D], bf16, kind="ExternalInput")
    idx_d = nc.dram_tensor("idx16", [P, S_total // 16], mybir.dt.int16, kind="ExternalInput")
    dst_d = nc.dram_tensor("dstl", [P, S_total // 128], f32, kind="ExternalInput")
    wt_d = nc.dram_tensor("wt", [D, D], bf16, kind="ExternalInput")
    b_d = nc.dram_tensor("bias", [1, D], bf16, kind="ExternalInput")
    out_d = nc.dram_tensor("out", [NG * P, G * D], bf16, kind="ExternalOutput")

    with tile.TileContext(nc) as tc, ExitStack() as ctx:
        const = ctx.enter_context(tc.tile_pool(name="const", bufs=1))
        gxp = ctx.enter_context(tc.tile_pool(name="gx", bufs=3))
        ohp = ctx.enter_context(tc.tile_pool(name="oh", bufs=3))
        xgp = ctx.enter_context(tc.tile_pool(name="xg", bufs=3))
        htp = ctx.enter_context(tc.tile_pool(name="ht", bufs=4))
        obp = ctx.enter_context(tc.tile_pool(name="ob", bufs=3))
        pag = ctx.enter_context(tc.tile_pool(name="pagg", bufs=4, space="PSUM"))
        pou = ctx.enter_context(tc.tile_pool(name="pout", bufs=4, space="PSUM"))

        idx_t = const.tile([P, S_total // 16], mybir.dt.int16)
        nc.sync.dma_start(out=idx_t[:], in_=idx_d[:])
        dst_t = const.tile([P, S_total // 128], f32)
        nc.sync.dma_start(out=dst_t[:], in_=dst_d[:])
        wt_t = const.tile([D, D], bf16)
        nc.sync.dma_start(out=wt_t[:], in_=wt_d[:])
        b_t = const.tile([1, D], bf16)
        nc.sync.dma_start(out=b_t[:], in_=b_d[:])
        ones_t = const.tile([1, D], bf16)
        nc.vector.memset(ones_t[:], 1.0)
        iota_i = const.tile([P, P], mybir.dt.int32)
        nc.gpsimd.iota(iota_i[:], pattern=[[1, P]], base=0, channel_multiplier=0)
        iota_f = const.tile([P, P], bf16)
        nc.vector.tensor_copy(out=iota_f[:], in_=iota_i[:])
        iota_p = const.tile([P, 1], mybir.dt.int32)
        nc.gpsimd.iota(iota_p[:], pattern=[[1, 1]], base=0, channel_multiplier=1)
        ident_t = const.tile([P, P], bf16)
        nc.vector.tensor_tensor(
            out=ident_t[:],
            in0=iota_i[:],
            in1=iota_p[:].to_broadcast([P, P]),
            op=mybir.AluOpType.is_equal,
        )

        gather_count = 0
        for _rep in range(repeat):
            for gi, (ta, tb) in enumerate(groups):
                Bg = int(Btq[ta:tb, :].sum())
                c0g = int(slot_start[ta, 0]) // P  # first chunk col of group
                gx = gxp.tile([P, Bg, P], bf16, tag="gx")
                oh = ohp.tile([P, Bg, P], bf16, tag="oh")
                for q in range(NQ):
                    Bgq = int(Btq[ta:tb, q].sum())
                    if Bgq == 0:
                        continue
                    c0 = int(slot_start[ta, q]) // P
                    b0 = c0 - c0g
                    # HW ring limit: 1024 indices (8 blocks) per dma_gather
                    for sb in range(0, Bgq, 8):
                        nb = min(8, Bgq - sb)
                        nidx = nb * P
                        nc.gpsimd.dma_gather(
                            gx[:, b0 + sb : b0 + sb + nb, :],
                            x_d[int(qstart[q]) : int(qstart[q] + qsizes[q]), :],
                            idx_t[:, (c0 + sb) * 8 : (c0 + sb) * 8 + nidx // 16],
                            nidx,
                            nidx,
                            D,
                            queue_num=gather_count % 4,
                        )
                        gather_count += 1
                    # one-hot per chunk via tensor_scalar (per-partition dst
                    # scalar) — keeps innermost strides 1 so DVE 2x/4x perf
                    # modes engage, unlike a broadcast tensor_tensor
                    for bb in range(Bgq):
                        nc.vector.tensor_scalar(
                            out=oh[:, b0 + bb, :],
                            in0=iota_f[:],
                            scalar1=dst_t[:, c0 + bb : c0 + bb + 1],
                            scalar2=None,
                            op0=mybir.AluOpType.is_equal,
                        )
                xg_t = xgp.tile([P, G * D], bf16, tag="xg")
                nc.sync.dma_start(out=xg_t[:], in_=xg_d[gi * P : (gi + 1) * P, :])
                ob = obp.tile([P, G * D], bf16, tag="ob")
                for i, t in enumerate(range(ta, tb)):
                    psum = pag.tile([P, P], f32, space="PSUM", tag="pagg")
                    # self term: psum[feat, slot] = x_tile^T via identity
                    nc.tensor.matmul(
                        out=psum[:],
                        lhsT=xg_t[:, i * D : (i + 1) * D],
                        rhs=ident_t[:],
                        start=True,
                        stop=False,
                    )
                    chunks = [(q, b) for q in range(NQ) for b in range(int(Btq[t, q]))]
                    for j, (q, b) in enumerate(chunks):
                        boff = (int(slot_start[t, q]) // P) - c0g + b
                        nc.tensor.matmul(
                            out=psum[:],
                            lhsT=gx[:, boff, :],
                            rhs=oh[:, boff, :],
                            start=False,
                            stop=(j == len(chunks) - 1),
                        )
                    ht = htp.tile([P, P], bf16, tag="ht")
                    nc.scalar.activation(ht[:], psum[:], mybir.ActivationFunctionType.Copy)
                    po = pou.tile([P, P], f32, space="PSUM", tag="pout")
                    nc.tensor.matmul(out=po[:], lhsT=ht[:], rhs=wt_t[:], start=True, stop=False)
                    nc.tensor.matmul(out=po[:], lhsT=ones_t[:], rhs=b_t[:], start=False, stop=True)
                    nc.scalar.activation(
                        ob[:, i * D : (i + 1) * D], po[:],
                        mybir.ActivationFunctionType.Relu,
                    )
                nc.sync.dma_start(out=out_d[gi * P : (gi + 1) * P, :], in_=ob[:])
    nc.compile()
    return nc


def _make_in_maps(x, W, b, pre):
    x = np.asarray(x, dtype=_f32)
    W = np.asarray(W, dtype=_f32)
    b = np.asarray(b, dtype=_f32)
    x_bf16_nat = x.astype(_bf16)
    # device x table is permuted so quarter q is a contiguous row range
    x_bf = np.empty((N, D), _bf16)
    x_bf[pre["xrow_of_node"]] = x_bf16_nat
    wt = np.ascontiguousarray(W.T.astype(_bf16))
    brow = np.ascontiguousarray(b.reshape(1, D).astype(_bf16))
    node_of = pre["node_of"]
    in_maps = []
    for c in range(NC):
        nidx = np.where(node_of[c] < 0, 0, node_of[c])
        # xg[g*128 + slot, i*128 + feat] = x[node at tile (g*G+i), slot]
        xg = np.ascontiguousarray(
            x_bf16_nat[nidx]               # [SLOTS, D]
            .reshape(NG, G, P, D)          # [g, i, slot, feat]
            .transpose(0, 2, 1, 3)         # [g, slot, i, feat]
            .reshape(NG * P, G * D)
        )
        in_maps.append(
            {
                "x": x_bf,
                "xg": xg,
                "idx16": np.ascontiguousarray(pre["idx16"][c]),
                "dstl": np.ascontiguousarray(pre["dstl"][c]),
                "wt": wt,
                "bias": brow,
            }
        )
    return in_maps


def _prepare(x, edge_index, W, b, repeat=1):
    pre = _preprocess(edge_index)
    nc = _build_program(
        pre["Btq"], pre["slot_start"], pre["S_total"], pre["groups"],
        pre["qstart"], pre["qsizes"], repeat=repeat,
    )
    in_maps = _make_in_maps(x, W, b, pre)
    return nc, in_maps, pre["node_of"]


def _assemble(results, node_of):
    out = np.empty((N, D), _f32)
    for c in range(NC):
        oc = (
            np.asarray(results[c]["out"])
            .astype(_f32)
            .reshape(NG, P, G, D)
            .transpose(0, 2, 1, 3)      # [g, i, slot, feat]
            .reshape(SLOTS, D)
        )
        m = node_of[c] >= 0
        out[node_of[c][m]] = oc[m]
    return out


def kernel(x, edge_index, W, b):
    from concourse.bass_utils import run_bass_kernel_spmd

    nc, in_maps, node_of = _prepare(x, edge_index, W, b)
    res = run_bass_kernel_spmd(nc, in_maps, core_ids=list(range(NC)))
    return _assemble(res.results, node_of)
